# revision 12
# baseline (speedup 1.0000x reference)
"""CrystalGraphConv message-passing kernel for 8 Trainium2 NeuronCores.

Sharding: destination nodes split across the 8 cores (12500 each); the
node-feature table is replicated. Each core computes the full transformed
table x_t = x @ weight on-device, then processes its in-edges in 128-node
destination blocks (a fixed quota of TPB tiles of 128 edge slots per block,
host-padded). Per tile: an indirect DMA gathers 128 x_t rows; edge weights
sigmoid(edge_attr * ew_W + ew_b) are applied on-chip; a one-hot selection
matrix (is_equal against an iota row) turns the per-block segment-sum into
PSUM-accumulated matmuls. Block results (+bias) stream straight to the
core's output shard; the host concatenates the 8 shards.
"""
import os
import sys
sys.path.insert(0, "/opt/trn_rl_repo")
import numpy as np

N_NODES = 100000
N_EDGES = 1600000
D = 64
N_CORES = 8
NODES_PER_CORE = N_NODES // N_CORES      # 12500
NBLK = (NODES_PER_CORE + 127) // 128     # 98 (last block has 84 rows)
TPB = 18                                 # tiles (128 edge slots) per block
NT = NBLK * TPB
NSLOT = NT * 128
GATHER_SPLIT = os.environ.get("GATHER_SPLIT", "1") == "1"
NO_GATHER = os.environ.get("NO_GATHER", "0") == "1"
EWB_ZERO = True  # set per-call from the actual ew_b input in make_inputs()

_cache = {}


def build_nc(n_cores=N_CORES, n_nodes=N_NODES, nodes_per_core=NODES_PER_CORE,
             nblk=NBLK, tpb=TPB, debug=False):
    import concourse.bacc as bacc
    import concourse.bass as bass
    import concourse.mybir as mybir
    import concourse.tile as tile

    F32 = mybir.dt.float32
    I32 = mybir.dt.int32
    nt = nblk * tpb

    nc = bacc.Bacc("TRN2", target_bir_lowering=False, debug=debug,
                   num_devices=n_cores)

    xT_d = nc.dram_tensor("xT", [D, n_nodes], F32, kind="ExternalInput")
    w_d = nc.dram_tensor("w", [D, D], F32, kind="ExternalInput")
    wrep_d = nc.dram_tensor("wrep", [128, D], F32, kind="ExternalInput")
    brep_d = nc.dram_tensor("brep", [128, D], F32, kind="ExternalInput")
    biasrep_d = nc.dram_tensor("biasrep", [128, D], F32, kind="ExternalInput")
    idx_d = nc.dram_tensor("idxs", [128, nt], I32, kind="ExternalInput")
    a_d = nc.dram_tensor("attr", [128, nt], F32, kind="ExternalInput")
    dl_d = nc.dram_tensor("dl", [128, nt], F32, kind="ExternalInput")
    out_d = nc.dram_tensor("out", [nodes_per_core, D], F32, kind="ExternalOutput")
    table_d = nc.dram_tensor("xt_table", [n_nodes, D], F32)  # internal

    with tile.TileContext(nc) as tc:
        with tc.tile_pool(name="const", bufs=1) as cpool, \
             tc.tile_pool(name="xin", bufs=3) as xinp, \
             tc.tile_pool(name="xtw", bufs=6) as xtp, \
             tc.tile_pool(name="idxt", bufs=24) as idxp, \
             tc.tile_pool(name="gat", bufs=6) as gat, \
             tc.tile_pool(name="pb", bufs=4) as pbp, \
             tc.tile_pool(name="fl", bufs=6) as flp, \
             tc.tile_pool(name="ps", bufs=4, space="PSUM") as psp:

            # ---------- phase 0: x_t = x @ weight -> internal table
            w_sb = cpool.tile([D, D], F32)
            nc.sync.dma_start(w_sb[:], w_d[:])
            GROUP = 2048
            n_groups = (n_nodes + GROUP - 1) // GROUP
            for g in range(n_groups):
                lo = g * GROUP
                m = min(GROUP, n_nodes - lo)
                ntile = (m + 127) // 128
                xts = xinp.tile([D, GROUP], F32, tag="xts")
                nc.sync.dma_start(xts[:, :m], xT_d[:, lo:lo + m])
                wide = xtp.tile([128, GROUP // 128, D], F32, tag="wide")
                for u in range(ntile):
                    mu = min(128, m - u * 128)
                    ps = psp.tile([128, D], F32, tag="xtps")
                    nc.tensor.matmul(ps[:mu, :], xts[:, u * 128:u * 128 + mu],
                                     w_sb[:], start=True, stop=True)
                    nc.vector.tensor_copy(wide[:mu, u, :], ps[:mu, :])
                if m % 128 == 0:
                    nc.sync.dma_start(
                        table_d[lo:lo + m, :].rearrange("(t p) c -> p t c", p=128),
                        wide[:, :ntile, :])
                else:
                    for u in range(ntile):
                        mu = min(128, m - u * 128)
                        nc.sync.dma_start(
                            table_d[lo + u * 128:lo + u * 128 + mu, :],
                            wide[:mu, u, :])

            # ---------- constants
            wrep = cpool.tile([128, D], F32)
            nc.sync.dma_start(wrep[:], wrep_d[:])
            brep = cpool.tile([128, D], F32)
            nc.sync.dma_start(brep[:], brep_d[:])
            biasrep = cpool.tile([128, D], F32)
            nc.sync.dma_start(biasrep[:], biasrep_d[:])
            iota_i = cpool.tile([128, 128], I32)
            nc.gpsimd.iota(iota_i[:], pattern=[[1, 128]], base=0,
                           channel_multiplier=0)
            iota_f = cpool.tile([128, 128], F32)
            nc.vector.tensor_copy(iota_f[:], iota_i[:])
            idx_slab = cpool.tile([128, nt], I32)
            nc.sync.dma_start(idx_slab[:], idx_d[:])
            a_slab = cpool.tile([128, nt], F32)
            nc.sync.dma_start(a_slab[:], a_d[:])
            dl_slab = cpool.tile([128, nt], F32)
            nc.sync.dma_start(dl_slab[:], dl_d[:])

            # ---------- main loop: one destination block at a time
            for b in range(nblk):
                t0 = b * tpb
                gt = gat.tile([128, tpb, D], F32, tag="gt")
                for ti in range(tpb):
                    off_ap = idx_slab[:, t0 + ti:t0 + ti + 1]
                    if NO_GATHER:
                        continue
                    if GATHER_SPLIT:
                        # gather into a private tile from a deep pool so
                        # consecutive gathers never WAW-serialize, then a
                        # cheap DVE copy assembles the batched tile
                        g1 = idxp.tile([128, D], F32, tag="g1")
                        nc.gpsimd.indirect_dma_start(
                            out=g1[:], out_offset=None, in_=table_d[:],
                            in_offset=bass.IndirectOffsetOnAxis(ap=off_ap, axis=0))
                        nc.vector.tensor_copy(gt[:, ti, :], g1[:])
                    else:
                        nc.gpsimd.indirect_dma_start(
                            out=gt[:, ti, :], out_offset=None, in_=table_d[:],
                            in_offset=bass.IndirectOffsetOnAxis(ap=off_ap, axis=0))
                wbuf = gat.tile([128, tpb, D], F32, tag="wbuf")
                a_b = a_slab[:, t0:t0 + tpb].unsqueeze(2).broadcast_to([128, tpb, D])
                w_b = wrep[:].unsqueeze(1).broadcast_to([128, tpb, D])
                nc.vector.tensor_tensor(wbuf[:], a_b, w_b, mybir.AluOpType.mult)
                if not EWB_ZERO:
                    b_b = brep[:].unsqueeze(1).broadcast_to([128, tpb, D])
                    nc.vector.tensor_tensor(wbuf[:], wbuf[:], b_b, mybir.AluOpType.add)
                nc.scalar.activation(wbuf[:], wbuf[:],
                                     mybir.ActivationFunctionType.Sigmoid)
                nc.vector.tensor_tensor(gt[:], gt[:], wbuf[:], mybir.AluOpType.mult)
                pb = pbp.tile([128, tpb, 128], F32, tag="pb")
                dl_b = dl_slab[:, t0:t0 + tpb].unsqueeze(2).broadcast_to([128, tpb, 128])
                io_b = iota_f[:].unsqueeze(1).broadcast_to([128, tpb, 128])
                nc.vector.tensor_tensor(pb[:], dl_b, io_b, mybir.AluOpType.is_equal)
                ps = psp.tile([128, D], F32, tag="blkps")
                for ti in range(tpb):
                    nc.tensor.matmul(ps[:], pb[:, ti, :], gt[:, ti, :],
                                     start=(ti == 0), stop=(ti == tpb - 1))
                fl = flp.tile([128, D], F32, tag="fl")
                nc.vector.scalar_tensor_tensor(
                    fl[:], ps[:], 1.0, biasrep[:],
                    mybir.AluOpType.mult, mybir.AluOpType.add)
                lo = b * 128
                hi = min(lo + 128, nodes_per_core)
                nc.sync.dma_start(out_d[lo:hi, :], fl[:hi - lo, :])

    nc.compile()
    return nc


def _get_runner():
    key = f"r{EWB_ZERO}"
    if key in _cache:
        return _cache[key]
    import jax
    from jax.sharding import Mesh, PartitionSpec
    from jax.experimental.shard_map import shard_map
    import concourse.mybir as mybir
    from concourse.bass2jax import (_bass_exec_p, install_neuronx_cc_hook,
                                    partition_id_tensor)

    nc = build_nc()
    install_neuronx_cc_hook()
    in_names, out_names, out_avals, zero_outs = [], [], [], []
    pname = nc.partition_id_tensor.name if nc.partition_id_tensor else None
    for alloc in nc.m.functions[0].allocations:
        if not isinstance(alloc, mybir.MemoryLocationSet):
            continue
        name = alloc.memorylocations[0].name
        if alloc.kind == "ExternalInput":
            if pname is None or name != pname:
                in_names.append(name)
        elif alloc.kind == "ExternalOutput":
            shape = tuple(alloc.tensor_shape)
            dtype = mybir.dt.np(alloc.dtype)
            out_names.append(name)
            out_avals.append(jax.core.ShapedArray(shape, dtype))
            zero_outs.append(np.zeros(shape, dtype))
    n_params, n_outs = len(in_names), len(out_avals)
    all_names = in_names + out_names + ([pname] if pname else [])
    donate = tuple(range(n_params, n_params + n_outs))

    def _body(*args):
        operands = list(args)
        if pname is not None:
            operands.append(partition_id_tensor())
        outs = _bass_exec_p.bind(
            *operands, out_avals=tuple(out_avals), in_names=tuple(all_names),
            out_names=tuple(out_names), lowering_input_output_aliases=(),
            sim_require_finite=True, sim_require_nnan=True, nc=nc)
        return tuple(outs)

    devices = jax.devices()[:N_CORES]
    mesh = Mesh(np.asarray(devices), ("core",))
    fn = jax.jit(
        shard_map(_body, mesh=mesh,
                  in_specs=(PartitionSpec("core"),) * (n_params + n_outs),
                  out_specs=(PartitionSpec("core"),) * n_outs,
                  check_rep=False),
        donate_argnums=donate, keep_unused=True)
    _cache[key] = (fn, in_names, out_names, out_avals, zero_outs)
    return _cache[key]


def shard_edges(edge_index, edge_attr, n_cores=N_CORES,
                nodes_per_core=NODES_PER_CORE, nblk=NBLK, tpb=TPB):
    """Host-side slot assignment -> per-core [128, NT] slabs.
    Slot s in tile t sits at partition s % 128 (edge slot = (p, t))."""
    src = np.asarray(edge_index[0], dtype=np.int64)
    dst = np.asarray(edge_index[1], dtype=np.int64)
    ea = np.asarray(edge_attr).reshape(-1).astype(np.float32)
    nt = nblk * tpb
    nslot = nt * 128
    core = dst // nodes_per_core
    idx_slabs, a_slabs, dl_slabs = [], [], []
    for k in range(n_cores):
        sel = np.nonzero(core == k)[0]
        d_loc = dst[sel] - k * nodes_per_core
        blk = d_loc // 128
        order = np.argsort(blk, kind="stable")
        sel = sel[order]
        blk = blk[order]
        d_in_blk = (d_loc[order] % 128).astype(np.float32)
        counts = np.bincount(blk, minlength=nblk)
        if counts.max() > tpb * 128:
            raise RuntimeError(f"block overflow: {counts.max()} > {tpb * 128}")
        idx = np.zeros(nslot, np.int32)
        att = np.zeros(nslot, np.float32)
        dl = np.full(nslot, -1.0, np.float32)
        starts = np.concatenate([[0], np.cumsum(counts)[:-1]])
        pos_in_blk = np.arange(len(sel)) - starts[blk]
        slot = blk * (tpb * 128) + pos_in_blk
        idx[slot] = src[sel].astype(np.int32)
        att[slot] = ea[sel]
        dl[slot] = d_in_blk
        idx_slabs.append(idx.reshape(nt, 128).T.copy())
        a_slabs.append(att.reshape(nt, 128).T.copy())
        dl_slabs.append(dl.reshape(nt, 128).T.copy())
    return idx_slabs, a_slabs, dl_slabs


def make_inputs(x, edge_index, edge_attr, weight, ew_W, ew_b, bias):
    global EWB_ZERO
    EWB_ZERO = not np.any(np.asarray(ew_b))
    idx_slabs, a_slabs, dl_slabs = shard_edges(edge_index, edge_attr)
    xT = np.ascontiguousarray(np.asarray(x, np.float32).T)
    weight = np.asarray(weight, np.float32)
    wrep = np.tile(np.asarray(ew_W, np.float32).reshape(1, D), (128, 1))
    brep = np.tile(np.asarray(ew_b, np.float32).reshape(1, D), (128, 1))
    biasrep = np.tile(np.asarray(bias, np.float32).reshape(1, D), (128, 1))
    return {
        "xT": [xT] * N_CORES, "w": [weight] * N_CORES,
        "wrep": [wrep] * N_CORES, "brep": [brep] * N_CORES,
        "biasrep": [biasrep] * N_CORES,
        "idxs": idx_slabs, "attr": a_slabs, "dl": dl_slabs,
    }


def stage_inputs(per_core):
    """device_put the concatenated per-core inputs once; reusable token."""
    import jax
    fn, in_names, out_names, out_avals, zero_outs = _get_runner()
    concat_in = [np.concatenate([np.asarray(per_core[n][c])
                                 for c in range(N_CORES)], axis=0)
                 for n in in_names]
    return [jax.device_put(a) for a in concat_in]


def run_staged(staged, fetch=True):
    import jax.numpy as jnp
    fn, in_names, out_names, out_avals, zero_outs = _get_runner()
    zeros = [jnp.zeros((N_CORES * z.shape[0], *z.shape[1:]), z.dtype)
             for z in zero_outs]
    outs = fn(*staged, *zeros)
    out_idx = out_names.index("out")
    if not fetch:
        outs[out_idx].block_until_ready()
        return None
    return np.asarray(outs[out_idx]).reshape(N_CORES * NODES_PER_CORE, D)


def run_prepared(per_core):
    return run_staged(stage_inputs(per_core))


def kernel(x, edge_index, edge_attr, weight, ew_W, ew_b, bias):
    per_core = make_inputs(x, edge_index, edge_attr, weight, ew_W, ew_b, bias)
    return run_prepared(per_core).astype(np.float32)


# revision 13
# speedup vs baseline: 1.3329x; 1.3329x over previous
"""CrystalGraphConv message-passing kernel for 8 Trainium2 NeuronCores.

Sharding: destination nodes split across the 8 cores (12500 each); the
node-feature table is replicated. Each core computes the full transformed
table x_t = x @ weight on-device, then processes its in-edges in 128-node
destination blocks (a fixed quota of TPB tiles of 128 edge slots per block,
host-padded). Per tile: an indirect DMA gathers 128 x_t rows; edge weights
sigmoid(edge_attr * ew_W + ew_b) are applied on-chip; a one-hot selection
matrix (is_equal against an iota row) turns the per-block segment-sum into
PSUM-accumulated matmuls. Block results (+bias) stream straight to the
core's output shard; the host concatenates the 8 shards.
"""
import os
import sys
sys.path.insert(0, "/opt/trn_rl_repo")
import numpy as np

N_NODES = 100000
N_EDGES = 1600000
D = 64
N_CORES = 8
NODES_PER_CORE = N_NODES // N_CORES      # 12500
NBLK = (NODES_PER_CORE + 127) // 128     # 98 (last block has 84 rows)
TPB = 18                                 # tiles (128 edge slots) per block
NT = NBLK * TPB
NSLOT = NT * 128
GATHER_SPLIT = os.environ.get("GATHER_SPLIT", "0") == "1"
NO_GATHER = os.environ.get("NO_GATHER", "0") == "1"
EWB_ZERO = True  # set per-call from the actual ew_b input in make_inputs()

_cache = {}


def build_nc(n_cores=N_CORES, n_nodes=N_NODES, nodes_per_core=NODES_PER_CORE,
             nblk=NBLK, tpb=TPB, debug=False):
    import concourse.bacc as bacc
    import concourse.bass as bass
    import concourse.mybir as mybir
    import concourse.tile as tile

    F32 = mybir.dt.float32
    I32 = mybir.dt.int32
    nt = nblk * tpb

    nc = bacc.Bacc("TRN2", target_bir_lowering=False, debug=debug,
                   num_devices=n_cores)

    xT_d = nc.dram_tensor("xT", [D, n_nodes], F32, kind="ExternalInput")
    w_d = nc.dram_tensor("w", [D, D], F32, kind="ExternalInput")
    wrep_d = nc.dram_tensor("wrep", [128, D], F32, kind="ExternalInput")
    brep_d = nc.dram_tensor("brep", [128, D], F32, kind="ExternalInput")
    biasrep_d = nc.dram_tensor("biasrep", [128, D], F32, kind="ExternalInput")
    idx_d = nc.dram_tensor("idxs", [128, nt], I32, kind="ExternalInput")
    a_d = nc.dram_tensor("attr", [128, nt], F32, kind="ExternalInput")
    dl_d = nc.dram_tensor("dl", [128, nt], F32, kind="ExternalInput")
    out_d = nc.dram_tensor("out", [nodes_per_core, D], F32, kind="ExternalOutput")
    table_d = nc.dram_tensor("xt_table", [n_nodes, D], F32)  # internal

    with tile.TileContext(nc) as tc:
        with tc.tile_pool(name="const", bufs=1) as cpool, \
             tc.tile_pool(name="xin", bufs=3) as xinp, \
             tc.tile_pool(name="xtw", bufs=6) as xtp, \
             tc.tile_pool(name="idxt", bufs=24) as idxp, \
             tc.tile_pool(name="gat", bufs=3) as gat, \
             tc.tile_pool(name="pb", bufs=2) as pbp, \
             tc.tile_pool(name="fl", bufs=6) as flp, \
             tc.tile_pool(name="ps", bufs=4, space="PSUM") as psp:

            # ---------- phase 0: x_t = x @ weight -> internal table
            w_sb = cpool.tile([D, D], F32)
            nc.sync.dma_start(w_sb[:], w_d[:])
            GROUP = 2048
            n_groups = (n_nodes + GROUP - 1) // GROUP
            for g in range(n_groups):
                lo = g * GROUP
                m = min(GROUP, n_nodes - lo)
                ntile = (m + 127) // 128
                xts = xinp.tile([D, GROUP], F32, tag="xts")
                nc.sync.dma_start(xts[:, :m], xT_d[:, lo:lo + m])
                wide = xtp.tile([128, GROUP // 128, D], F32, tag="wide")
                for u in range(ntile):
                    mu = min(128, m - u * 128)
                    ps = psp.tile([128, D], F32, tag="xtps")
                    nc.tensor.matmul(ps[:mu, :], xts[:, u * 128:u * 128 + mu],
                                     w_sb[:], start=True, stop=True)
                    nc.vector.tensor_copy(wide[:mu, u, :], ps[:mu, :])
                if m % 128 == 0:
                    nc.sync.dma_start(
                        table_d[lo:lo + m, :].rearrange("(t p) c -> p t c", p=128),
                        wide[:, :ntile, :])
                else:
                    for u in range(ntile):
                        mu = min(128, m - u * 128)
                        nc.sync.dma_start(
                            table_d[lo + u * 128:lo + u * 128 + mu, :],
                            wide[:mu, u, :])

            # ---------- constants
            wrep = cpool.tile([128, D], F32)
            nc.sync.dma_start(wrep[:], wrep_d[:])
            brep = cpool.tile([128, D], F32)
            nc.sync.dma_start(brep[:], brep_d[:])
            biasrep = cpool.tile([128, D], F32)
            nc.sync.dma_start(biasrep[:], biasrep_d[:])
            iota_i = cpool.tile([128, 128], I32)
            nc.gpsimd.iota(iota_i[:], pattern=[[1, 128]], base=0,
                           channel_multiplier=0)
            iota_f = cpool.tile([128, 128], F32)
            nc.vector.tensor_copy(iota_f[:], iota_i[:])
            idx_slab = cpool.tile([128, nt], I32)
            nc.sync.dma_start(idx_slab[:], idx_d[:])
            a_slab = cpool.tile([128, nt], F32)
            nc.sync.dma_start(a_slab[:], a_d[:])
            dl_slab = cpool.tile([128, nt], F32)
            nc.sync.dma_start(dl_slab[:], dl_d[:])

            # ---------- main loop: super-groups of SG blocks
            SG = 4          # blocks per weight/sigmoid batch
            PG = 2          # blocks per one-hot batch
            for s0 in range(0, nblk, SG):
                sgn = min(SG, nblk - s0)
                tw0 = s0 * tpb
                twn = sgn * tpb
                # batched edge weights for sgn blocks
                wbuf = gat.tile([128, SG * tpb, D], F32, tag="wbuf")
                a_b = a_slab[:, tw0:tw0 + twn].unsqueeze(2).broadcast_to([128, twn, D])
                w_b = wrep[:].unsqueeze(1).broadcast_to([128, twn, D])
                nc.vector.tensor_tensor(wbuf[:, :twn, :], a_b, w_b, mybir.AluOpType.mult)
                if not EWB_ZERO:
                    b_b = brep[:].unsqueeze(1).broadcast_to([128, twn, D])
                    nc.vector.tensor_tensor(wbuf[:, :twn, :], wbuf[:, :twn, :], b_b,
                                            mybir.AluOpType.add)
                nc.scalar.activation(wbuf[:, :twn, :], wbuf[:, :twn, :],
                                     mybir.ActivationFunctionType.Sigmoid)
                for p0 in range(s0, s0 + sgn, PG):
                    pgn = min(PG, s0 + sgn - p0)
                    tp0 = p0 * tpb
                    tpn = pgn * tpb
                    pb = pbp.tile([128, PG * tpb, 128], F32, tag="pb")
                    dl_b = dl_slab[:, tp0:tp0 + tpn].unsqueeze(2).broadcast_to([128, tpn, 128])
                    io_b = iota_f[:].unsqueeze(1).broadcast_to([128, tpn, 128])
                    nc.vector.tensor_tensor(pb[:, :tpn, :], dl_b, io_b,
                                            mybir.AluOpType.is_equal)
                    for b in range(p0, p0 + pgn):
                        t0 = b * tpb
                        gt = gat.tile([128, tpb, D], F32, tag="gt")
                        for ti in range(tpb):
                            off_ap = idx_slab[:, t0 + ti:t0 + ti + 1]
                            nc.gpsimd.indirect_dma_start(
                                out=gt[:, ti, :], out_offset=None, in_=table_d[:],
                                in_offset=bass.IndirectOffsetOnAxis(ap=off_ap, axis=0))
                        woff = (b - s0) * tpb
                        nc.vector.tensor_tensor(gt[:], gt[:],
                                                wbuf[:, woff:woff + tpb, :],
                                                mybir.AluOpType.mult)
                        ps = psp.tile([128, D], F32, tag="blkps")
                        poff = (b - p0) * tpb
                        for ti in range(tpb):
                            nc.tensor.matmul(ps[:], pb[:, poff + ti, :], gt[:, ti, :],
                                             start=(ti == 0), stop=(ti == tpb - 1))
                        fl = flp.tile([128, D], F32, tag="fl")
                        nc.vector.scalar_tensor_tensor(
                            fl[:], ps[:], 1.0, biasrep[:],
                            mybir.AluOpType.mult, mybir.AluOpType.add)
                        lo = b * 128
                        hi = min(lo + 128, nodes_per_core)
                        nc.sync.dma_start(out_d[lo:hi, :], fl[:hi - lo, :])

    nc.compile()
    return nc


def _get_runner():
    key = f"r{EWB_ZERO}"
    if key in _cache:
        return _cache[key]
    import jax
    from jax.sharding import Mesh, PartitionSpec
    from jax.experimental.shard_map import shard_map
    import concourse.mybir as mybir
    from concourse.bass2jax import (_bass_exec_p, install_neuronx_cc_hook,
                                    partition_id_tensor)

    nc = build_nc()
    install_neuronx_cc_hook()
    in_names, out_names, out_avals, zero_outs = [], [], [], []
    pname = nc.partition_id_tensor.name if nc.partition_id_tensor else None
    for alloc in nc.m.functions[0].allocations:
        if not isinstance(alloc, mybir.MemoryLocationSet):
            continue
        name = alloc.memorylocations[0].name
        if alloc.kind == "ExternalInput":
            if pname is None or name != pname:
                in_names.append(name)
        elif alloc.kind == "ExternalOutput":
            shape = tuple(alloc.tensor_shape)
            dtype = mybir.dt.np(alloc.dtype)
            out_names.append(name)
            out_avals.append(jax.core.ShapedArray(shape, dtype))
            zero_outs.append(np.zeros(shape, dtype))
    n_params, n_outs = len(in_names), len(out_avals)
    all_names = in_names + out_names + ([pname] if pname else [])
    donate = tuple(range(n_params, n_params + n_outs))

    def _body(*args):
        operands = list(args)
        if pname is not None:
            operands.append(partition_id_tensor())
        outs = _bass_exec_p.bind(
            *operands, out_avals=tuple(out_avals), in_names=tuple(all_names),
            out_names=tuple(out_names), lowering_input_output_aliases=(),
            sim_require_finite=True, sim_require_nnan=True, nc=nc)
        return tuple(outs)

    devices = jax.devices()[:N_CORES]
    mesh = Mesh(np.asarray(devices), ("core",))
    fn = jax.jit(
        shard_map(_body, mesh=mesh,
                  in_specs=(PartitionSpec("core"),) * (n_params + n_outs),
                  out_specs=(PartitionSpec("core"),) * n_outs,
                  check_rep=False),
        donate_argnums=donate, keep_unused=True)
    _cache[key] = (fn, in_names, out_names, out_avals, zero_outs)
    return _cache[key]


def shard_edges(edge_index, edge_attr, n_cores=N_CORES,
                nodes_per_core=NODES_PER_CORE, nblk=NBLK, tpb=TPB):
    """Host-side slot assignment -> per-core [128, NT] slabs.
    Slot s in tile t sits at partition s % 128 (edge slot = (p, t))."""
    src = np.asarray(edge_index[0], dtype=np.int64)
    dst = np.asarray(edge_index[1], dtype=np.int64)
    ea = np.asarray(edge_attr).reshape(-1).astype(np.float32)
    nt = nblk * tpb
    nslot = nt * 128
    core = dst // nodes_per_core
    idx_slabs, a_slabs, dl_slabs = [], [], []
    for k in range(n_cores):
        sel = np.nonzero(core == k)[0]
        d_loc = dst[sel] - k * nodes_per_core
        blk = d_loc // 128
        order = np.argsort(blk, kind="stable")
        sel = sel[order]
        blk = blk[order]
        d_in_blk = (d_loc[order] % 128).astype(np.float32)
        counts = np.bincount(blk, minlength=nblk)
        if counts.max() > tpb * 128:
            raise RuntimeError(f"block overflow: {counts.max()} > {tpb * 128}")
        idx = np.zeros(nslot, np.int32)
        att = np.zeros(nslot, np.float32)
        dl = np.full(nslot, -1.0, np.float32)
        starts = np.concatenate([[0], np.cumsum(counts)[:-1]])
        pos_in_blk = np.arange(len(sel)) - starts[blk]
        slot = blk * (tpb * 128) + pos_in_blk
        idx[slot] = src[sel].astype(np.int32)
        att[slot] = ea[sel]
        dl[slot] = d_in_blk
        idx_slabs.append(idx.reshape(nt, 128).T.copy())
        a_slabs.append(att.reshape(nt, 128).T.copy())
        dl_slabs.append(dl.reshape(nt, 128).T.copy())
    return idx_slabs, a_slabs, dl_slabs


def make_inputs(x, edge_index, edge_attr, weight, ew_W, ew_b, bias):
    global EWB_ZERO
    EWB_ZERO = not np.any(np.asarray(ew_b))
    idx_slabs, a_slabs, dl_slabs = shard_edges(edge_index, edge_attr)
    xT = np.ascontiguousarray(np.asarray(x, np.float32).T)
    weight = np.asarray(weight, np.float32)
    wrep = np.tile(np.asarray(ew_W, np.float32).reshape(1, D), (128, 1))
    brep = np.tile(np.asarray(ew_b, np.float32).reshape(1, D), (128, 1))
    biasrep = np.tile(np.asarray(bias, np.float32).reshape(1, D), (128, 1))
    return {
        "xT": [xT] * N_CORES, "w": [weight] * N_CORES,
        "wrep": [wrep] * N_CORES, "brep": [brep] * N_CORES,
        "biasrep": [biasrep] * N_CORES,
        "idxs": idx_slabs, "attr": a_slabs, "dl": dl_slabs,
    }


def stage_inputs(per_core):
    """device_put the concatenated per-core inputs once; reusable token."""
    import jax
    fn, in_names, out_names, out_avals, zero_outs = _get_runner()
    concat_in = [np.concatenate([np.asarray(per_core[n][c])
                                 for c in range(N_CORES)], axis=0)
                 for n in in_names]
    return [jax.device_put(a) for a in concat_in]


def run_staged(staged, fetch=True):
    import jax.numpy as jnp
    fn, in_names, out_names, out_avals, zero_outs = _get_runner()
    zeros = [jnp.zeros((N_CORES * z.shape[0], *z.shape[1:]), z.dtype)
             for z in zero_outs]
    outs = fn(*staged, *zeros)
    out_idx = out_names.index("out")
    if not fetch:
        outs[out_idx].block_until_ready()
        return None
    return np.asarray(outs[out_idx]).reshape(N_CORES * NODES_PER_CORE, D)


def run_prepared(per_core):
    return run_staged(stage_inputs(per_core))


def kernel(x, edge_index, edge_attr, weight, ew_W, ew_b, bias):
    per_core = make_inputs(x, edge_index, edge_attr, weight, ew_W, ew_b, bias)
    return run_prepared(per_core).astype(np.float32)
